# revision 17
# baseline (speedup 1.0000x reference)
"""Self-contained DKVMN Bass kernel (graded entry: kernel(**inputs)) for TRN2, 8-core data-parallel over batch.

Time-major layout, PE-assisted scan:
  l' = t*16 + b  (t-major flattened index per core; b in 0..15, t in 0..199)
  p = l' % 128 = (t%8)*16 + b ; j = l' // 128 = t // 8
  k_t/v_t    [k, l']           gathered embedding rows, transposed
  e_bT8/a_bT8[p=(t8,b), (j,k)] bf16 — sigmoid/tanh outputs in l'-partition
                               orientation; lhsT slices for the scan matmuls
  w_sb       [p=(t8,b), (j,v)] softmax weights; w_bf = bf16 cast
  Wd_dram    block-diagonal w staging: flat [S, 17408] bf16 where
             row t holds w[b,v] at offset b*1088 + v (b*1088 = b*1024+b*64,
             so the dense [16,1024] read at row-stride 1024 sees w[b] on the
             diagonal block b*64..b*64+64 and zeros elsewhere)
  Scan state M[k, (b,v)]  [128, 1024] f32 in SBUF
  Per step (PE): Wb = ones16^T @ Wd_t         (w broadcast over k)
                 X  = e_t^T @ Wd_t - 1        (w*e - 1)
                 WA = a_t^T @ Wd_t            (w*a)
  Per step (DVE, flat 2D APs): rp = M*Wb ; read = reduce_v(rp) ;
                 T1 = M*X ; M' = WA - T1
  reads_all  [k, l']           feeds f/p matmuls directly (no relayout)
"""

import numpy as np

import concourse.bacc as bacc
import concourse.bass as bass
import concourse.mybir as mybir
from concourse.tile import TileContext
from concourse.masks import make_identity

F32 = mybir.dt.float32
BF16 = mybir.dt.bfloat16
I32 = mybir.dt.int32
AX = mybir.AxisListType
ALU = mybir.AluOpType
ACTF = mybir.ActivationFunctionType

B, S, DK, DV, NQ = 128, 200, 128, 64, 10000
NC = 8
BL = B // NC          # 16 batches per core
L = BL * S            # 3200 lookups per core
NJ = L // 128         # 25 gather groups
CH = 400              # matmul free-dim chunk (<=512)
NCH = L // CH         # 8 chunks
WDROW = 17408         # 16*1088 per-step row in Wd staging
FB = BL * DV          # 1024 scan free size


def build_kernel(reps=1):
    nc = bacc.Bacc("TRN2", target_bir_lowering=False, debug=False, num_devices=NC)

    # ---- I/O ----
    qidx = nc.dram_tensor("qidx", [128, NJ], I32, kind="ExternalInput").ap()
    xidx = nc.dram_tensor("xidx", [128, NJ], I32, kind="ExternalInput").ap()
    k_emb = nc.dram_tensor("k_emb", [NQ, DK], F32, kind="ExternalInput").ap()
    v_emb = nc.dram_tensor("v_emb", [2 * NQ, DK], F32, kind="ExternalInput").ap()
    MkT = nc.dram_tensor("MkT", [DK, DV], F32, kind="ExternalInput").ap()
    eW = nc.dram_tensor("eW", [DK, DK], F32, kind="ExternalInput").ap()
    aW = nc.dram_tensor("aW", [DK, DK], F32, kind="ExternalInput").ap()
    fWr = nc.dram_tensor("fWr", [DK, DK], F32, kind="ExternalInput").ap()
    fWk = nc.dram_tensor("fWk", [DK, DK], F32, kind="ExternalInput").ap()
    pW = nc.dram_tensor("pW", [DK, 1], F32, kind="ExternalInput").ap()
    ebr = nc.dram_tensor("ebr", [1, DK], F32, kind="ExternalInput").ap()
    abr = nc.dram_tensor("abr", [1, DK], F32, kind="ExternalInput").ap()
    fb = nc.dram_tensor("fb", [DK, 1], F32, kind="ExternalInput").ap()
    pb = nc.dram_tensor("pb", [1, 1], F32, kind="ExternalInput").ap()
    M0c = nc.dram_tensor("M0c", [128, FB], F32, kind="ExternalInput").ap()
    out = nc.dram_tensor("out", [1, L], F32, kind="ExternalOutput").ap()

    # ---- DRAM scratch: block-diagonal w staging + e/a relayout bounce ----
    wd_dram = nc.dram_tensor("wd_scr", [S, WDROW], BF16).ap()
    e_scr = nc.dram_tensor("e_scr", [L, DK], BF16).ap()
    a_scr = nc.dram_tensor("a_scr", [L, DK], BF16).ap()
    w_scr = nc.dram_tensor("w_scr", [128, NJ * DV], BF16).ap()

    with TileContext(nc) as tc:
        with (
            tc.tile_pool(name="persist", bufs=1) as pp,
            tc.tile_pool(name="work", bufs=2) as wp,
        ):
            # ---------- params to SBUF ----------
            ident = pp.tile([128, 128], F32)
            make_identity(nc, ident[:])
            MkT_sb = pp.tile([DK, DV], F32)
            nc.sync.dma_start(out=MkT_sb[:], in_=MkT)
            eW_sb = pp.tile([DK, DK], F32)
            nc.sync.dma_start(out=eW_sb[:], in_=eW)
            aW_sb = pp.tile([DK, DK], F32)
            nc.sync.dma_start(out=aW_sb[:], in_=aW)
            fWr_sb = pp.tile([DK, DK], F32)
            nc.sync.dma_start(out=fWr_sb[:], in_=fWr)
            fWk_sb = pp.tile([DK, DK], F32)
            nc.sync.dma_start(out=fWk_sb[:], in_=fWk)
            pW_sb = pp.tile([DK, 1], F32)
            nc.sync.dma_start(out=pW_sb[:], in_=pW)
            ebr_sb = pp.tile([1, DK], F32)
            nc.sync.dma_start(out=ebr_sb[:], in_=ebr)
            abr_sb = pp.tile([1, DK], F32)
            nc.sync.dma_start(out=abr_sb[:], in_=abr)
            fb_sb = pp.tile([DK, 1], F32)
            nc.sync.dma_start(out=fb_sb[:], in_=fb)
            pb_sb = pp.tile([1, 1], F32)
            nc.sync.dma_start(out=pb_sb[:], in_=pb)
            qidx_sb = pp.tile([128, NJ], I32)
            nc.sync.dma_start(out=qidx_sb[:], in_=qidx)
            xidx_sb = pp.tile([128, NJ], I32)
            nc.sync.dma_start(out=xidx_sb[:], in_=xidx)

            # constants
            ones_col = pp.tile([1, 128], F32)
            nc.vector.memset(ones_col[:], 1.0)
            ones16 = pp.tile([16, 128], BF16)
            nc.vector.memset(ones16[:], 1.0)
            negones = pp.tile([1, 128], BF16)
            nc.vector.memset(negones[:], -1.0)
            onesrow = pp.tile([1, FB], BF16)
            nc.vector.memset(onesrow[:], 1.0)

            # ---------- persistent tiles (hoisted out of the reps loop) ----------
            zsb = pp.tile([128, 1700], BF16)
            nc.vector.memset(zsb[:], 0.0)
            wd_zero_view = wd_dram.rearrange("s r -> (s r)").rearrange(
                "(n p f) -> n p f", p=128, f=1700
            )
            k_t = pp.tile([128, L], F32)   # [k, l']
            w_sb = pp.tile([128, NJ * DV], F32)
            w_bf = pp.tile([128, NJ * DV], BF16)
            e_b16 = pp.tile([16, S * DK], BF16)
            a_b16 = pp.tile([16, S * DK], BF16)
            reads_all = pp.tile([128, L], F32)   # [k, l']
            pred = pp.tile([1, L], F32)

          # reps>1 repeats the whole pipeline for dispatch-free timing
          # (kernel output identical each rep)
          for rep in range(reps):
            # ---------- zero-fill Wd staging (16 x [128,1700] = S*WDROW) ----------
            for i in range(16):
                nc.sync.dma_start(out=wd_zero_view[i], in_=zsb[:])

            # ---------- gather + transpose ----------
            v_pool = tc.tile_pool(name=f"vt{rep}", bufs=1)
            vp = v_pool.__enter__()
            v_t = vp.tile([128, L], F32)   # [k, l']
            pre_psum = tc.tile_pool(name=f"pre_psum{rep}", bufs=2, space="PSUM")
            prp = pre_psum.__enter__()
            for j in range(NJ):
                ksl = wp.tile([128, 128], F32, tag="gk")
                nc.gpsimd.indirect_dma_start(
                    out=ksl[:],
                    out_offset=None,
                    in_=k_emb,
                    in_offset=bass.IndirectOffsetOnAxis(ap=qidx_sb[:, j : j + 1], axis=0),
                )
                tp = prp.tile([128, 128], F32, tag="tr")
                nc.tensor.transpose(out=tp[:], in_=ksl[:], identity=ident[:])
                nc.scalar.copy(out=k_t[:, j * 128 : (j + 1) * 128], in_=tp[:])

                vsl = wp.tile([128, 128], F32, tag="gv")
                nc.gpsimd.indirect_dma_start(
                    out=vsl[:],
                    out_offset=None,
                    in_=v_emb,
                    in_offset=bass.IndirectOffsetOnAxis(ap=xidx_sb[:, j : j + 1], axis=0),
                )
                tp2 = prp.tile([128, 128], F32, tag="tr")
                nc.tensor.transpose(out=tp2[:], in_=vsl[:], identity=ident[:])
                nc.scalar.copy(out=v_t[:, j * 128 : (j + 1) * 128], in_=tp2[:])

            # ---------- w = softmax(k @ Mk^T) in [p=(t8,b), (j,v)] ----------
            w_sb = pp.tile([128, NJ * DV], F32)
            for j in range(NJ):
                wps = prp.tile([128, DV], F32, tag="mmw")
                nc.tensor.matmul(
                    out=wps[:],
                    lhsT=k_t[:, j * 128 : (j + 1) * 128],
                    rhs=MkT_sb[:],
                    start=True,
                    stop=True,
                )
                negmax = wp.tile([128, 1], F32, tag="negmax")
                nc.vector.tensor_reduce(
                    out=negmax[:], in_=wps[:], axis=AX.X, op=ALU.max, negate=True
                )
                expt = wp.tile([128, DV], F32, tag="expt")
                sums = wp.tile([128, 1], F32, tag="sums")
                nc.scalar.activation(
                    out=expt[:], in_=wps[:], func=ACTF.Exp,
                    bias=negmax[:], accum_out=sums[:],
                )
                rsum = wp.tile([128, 1], F32, tag="rsum")
                nc.vector.reciprocal(out=rsum[:], in_=sums[:])
                nc.vector.tensor_scalar_mul(
                    w_sb[:, j * DV : (j + 1) * DV], expt[:], rsum[:, :1]
                )
            w_bf = pp.tile([128, NJ * DV], BF16)
            nc.scalar.copy(out=w_bf[:], in_=w_sb[:])

            # scatter w into the zeroed Wd staging (diagonal offsets) via a
            # DRAM bounce — DRAM-side strides are unrestricted
            nc.sync.dma_start(out=w_scr, in_=w_bf[:])
            wd_scatter = wd_dram.rearrange(
                "(j t8) (b r) -> t8 b j r", t8=8, r=1088
            )[:, :, :, 0:DV]
            nc.sync.dma_start(
                out=wd_scatter,
                in_=w_scr.rearrange("(t8 b) (j v) -> t8 b j v", t8=8, v=DV),
            )

            # ---------- e/a in [b, (t,k)] bf16 (base-0 lhsT slices per step) ----------
            # chunk j covers l' = j*128..(j+1)*128 = t in [8j, 8j+8) x b; the
            # sigmoid/tanh output [p=(t8,b), k] is partition-remapped to
            # [b, (t,k)] via SBUF->SBUF DMA.
            e_b16 = pp.tile([16, S * DK], BF16)
            a_b16 = pp.tile([16, S * DK], BF16)
            for j in range(NJ):
                vch = v_t[:, j * 128 : (j + 1) * 128]
                eps = prp.tile([128, DK], F32, tag="mme")
                nc.tensor.matmul(out=eps[:], lhsT=vch, rhs=eW_sb[:], start=True, stop=False)
                nc.tensor.matmul(out=eps[:], lhsT=ones_col[:], rhs=ebr_sb[:], start=False, stop=True)
                etmp = wp.tile([128, DK], BF16, tag="etmp")
                nc.scalar.activation(out=etmp[:], in_=eps[:], func=ACTF.Sigmoid)
                # bounce through DRAM: SBUF partition remap is not a legal
                # single-DMA AP, but DRAM-side strides are unrestricted
                nc.sync.dma_start(
                    out=e_scr.rearrange("(j p) k -> j p k", p=128)[j], in_=etmp[:]
                )
                nc.sync.dma_start(
                    out=e_b16[:, j * 8 * DK : (j + 1) * 8 * DK].rearrange(
                        "b (t8 k) -> b t8 k", t8=8
                    ),
                    in_=e_scr.rearrange("(j t8 b) k -> j b t8 k", t8=8, b=16)[j],
                )
                aps = prp.tile([128, DK], F32, tag="mme")
                nc.tensor.matmul(out=aps[:], lhsT=vch, rhs=aW_sb[:], start=True, stop=False)
                nc.tensor.matmul(out=aps[:], lhsT=ones_col[:], rhs=abr_sb[:], start=False, stop=True)
                atmp = wp.tile([128, DK], BF16, tag="atmp")
                nc.scalar.activation(out=atmp[:], in_=aps[:], func=ACTF.Tanh)
                nc.sync.dma_start(
                    out=a_scr.rearrange("(j p) k -> j p k", p=128)[j], in_=atmp[:]
                )
                nc.sync.dma_start(
                    out=a_b16[:, j * 8 * DK : (j + 1) * 8 * DK].rearrange(
                        "b (t8 k) -> b t8 k", t8=8
                    ),
                    in_=a_scr.rearrange("(j t8 b) k -> j b t8 k", t8=8, b=16)[j],
                )

            pre_psum.__exit__(None, None, None)
            v_pool.__exit__(None, None, None)

            # ---------- the scan ----------
            reads_all = pp.tile([128, L], F32)   # [k, l']
            scan_psum = tc.tile_pool(name="scan_psum", bufs=1, space="PSUM")
            scp = scan_psum.__enter__()
            wd_pool = tc.tile_pool(name="wd", bufs=3)
            wdp = wd_pool.__enter__()
            sp = tc.tile_pool(name="state", bufs=2)
            stp = sp.__enter__()
            tmp_pool = tc.tile_pool(name="scantmp", bufs=2)
            tmp = tmp_pool.__enter__()

            M_cur = stp.tile([128, FB], F32, tag="M")
            nc.sync.dma_start(out=M_cur[:], in_=M0c)

            wd_load = wd_dram[:, 0 : 16 * FB].rearrange("s (b c) -> s b c", c=FB)

            for t in range(S):
                e_sl = e_b16[:, t * DK : (t + 1) * DK]
                a_sl = a_b16[:, t * DK : (t + 1) * DK]

                wd_sb = wdp.tile([16, FB], BF16, tag="wd")
                nc.sync.dma_start(out=wd_sb[:], in_=wd_load[t])

                # matmul out free dim is limited to one PSUM bank (512 f32):
                # write each [128,1024] operand in two bank-sized halves.
                wb = scp.tile([128, FB], F32, tag="wb")
                xx = scp.tile([128, FB], F32, tag="xx")
                wa = scp.tile([128, FB], F32, tag="wa")
                for h in range(2):
                    hs = slice(h * 512, (h + 1) * 512)
                    nc.tensor.matmul(out=wb[:, hs], lhsT=ones16[:], rhs=wd_sb[:, hs], start=True, stop=True)
                    nc.tensor.matmul(out=xx[:, hs], lhsT=negones[:], rhs=onesrow[:, hs], start=True, stop=False)
                    nc.tensor.matmul(out=xx[:, hs], lhsT=e_sl, rhs=wd_sb[:, hs], start=False, stop=True)
                    nc.tensor.matmul(out=wa[:, hs], lhsT=a_sl, rhs=wd_sb[:, hs], start=True, stop=True)

                rp = tmp.tile([128, FB], F32, tag="rp")
                nc.vector.tensor_tensor(out=rp[:], in0=M_cur[:], in1=wb[:], op=ALU.mult)
                nc.vector.tensor_reduce(
                    out=reads_all[:, t * 16 : (t + 1) * 16],
                    in_=rp[:].rearrange("p (b v) -> p b v", v=DV),
                    axis=AX.X,
                    op=ALU.add,
                )
                t1 = tmp.tile([128, FB], F32, tag="t1")
                nc.vector.tensor_tensor(out=t1[:], in0=M_cur[:], in1=xx[:], op=ALU.mult)
                M_new = stp.tile([128, FB], F32, tag="M")
                nc.vector.tensor_tensor(out=M_new[:], in0=wa[:], in1=t1[:], op=ALU.subtract)
                M_cur = M_new

            tmp_pool.__exit__(None, None, None)
            sp.__exit__(None, None, None)
            wd_pool.__exit__(None, None, None)
            scan_psum.__exit__(None, None, None)

            # ---------- f = tanh([reads, k] @ fW + fb); p = sigmoid(f@pW+pb) ----------
            fin_psum = tc.tile_pool(name="fin_psum", bufs=2, space="PSUM")
            fpp = fin_psum.__enter__()
            pred = pp.tile([1, L], F32)
            for c in range(NCH):
                cs = slice(c * CH, (c + 1) * CH)
                fps = fpp.tile([128, CH], F32, tag="mm")
                nc.tensor.matmul(out=fps[:], lhsT=fWr_sb[:], rhs=reads_all[:, cs], start=True, stop=False)
                nc.tensor.matmul(out=fps[:], lhsT=fWk_sb[:], rhs=k_t[:, cs], start=False, stop=True)
                f_sb = wp.tile([128, CH], F32, tag="fsb")
                nc.scalar.activation(out=f_sb[:], in_=fps[:], func=ACTF.Tanh, bias=fb_sb[:, :1])
                pps = fpp.tile([1, CH], F32, tag="mmp")
                nc.tensor.matmul(out=pps[:], lhsT=pW_sb[:], rhs=f_sb[:], start=True, stop=True)
                nc.scalar.activation(out=pred[:, cs], in_=pps[:], func=ACTF.Sigmoid, bias=pb_sb[:, :1])
            fin_psum.__exit__(None, None, None)

            nc.sync.dma_start(out=out, in_=pred[:])

    nc.compile()
    return nc


# ------------------------------------------------------------------
_CACHED = None


def _get_nc():
    global _CACHED
    if _CACHED is None:
        _CACHED = build_kernel()
    return _CACHED


def make_in_maps(question_seq, correct_seq, k_emb, v_emb, Mk, Mv0, fW, fb_, eW, eb_, aW, ab_, pW, pb_):
    q = np.asarray(question_seq).astype(np.int64)
    c = np.asarray(correct_seq).astype(np.int64)
    x = q + NQ * c

    shared = {
        "k_emb": np.ascontiguousarray(np.asarray(k_emb, np.float32)),
        "v_emb": np.ascontiguousarray(np.asarray(v_emb, np.float32)),
        "MkT": np.ascontiguousarray(np.asarray(Mk, np.float32).T),
        "eW": np.ascontiguousarray(np.asarray(eW, np.float32)),
        "aW": np.ascontiguousarray(np.asarray(aW, np.float32)),
        "fWr": np.ascontiguousarray(np.asarray(fW, np.float32)[:DK]),
        "fWk": np.ascontiguousarray(np.asarray(fW, np.float32)[DK:]),
        "pW": np.ascontiguousarray(np.asarray(pW, np.float32).reshape(DK, 1)),
        "ebr": np.ascontiguousarray(np.asarray(eb_, np.float32).reshape(1, DK)),
        "abr": np.ascontiguousarray(np.asarray(ab_, np.float32).reshape(1, DK)),
        "fb": np.ascontiguousarray(np.asarray(fb_, np.float32).reshape(DK, 1)),
        "pb": np.ascontiguousarray(np.asarray(pb_, np.float32).reshape(1, 1)),
    }
    # M0c[k, b*64+v] = Mv0[v, k]  (replicated over b)
    m0 = np.asarray(Mv0, np.float32).T          # [DK, DV]
    shared["M0c"] = np.ascontiguousarray(np.tile(m0, (1, BL)), np.float32)

    in_maps = []
    for core in range(NC):
        bs = slice(core * BL, (core + 1) * BL)
        # l' = t*16 + b  (t-major)
        qf = q[bs].T.reshape(-1)
        xf = x[bs].T.reshape(-1)
        qi = np.ascontiguousarray(qf.reshape(NJ, 128).T.astype(np.int32))  # [p, j]
        xi = np.ascontiguousarray(xf.reshape(NJ, 128).T.astype(np.int32))
        m = dict(shared)
        m["qidx"] = qi
        m["xidx"] = xi
        in_maps.append(m)
    return in_maps


def kernel(**inputs):
    from concourse.bass_utils import run_bass_kernel_spmd

    nc = _get_nc()
    in_maps = make_in_maps(
        inputs["question_seq"], inputs["correct_seq"], inputs["k_emb"],
        inputs["v_emb"], inputs["Mk"], inputs["Mv0"], inputs["fW"], inputs["fb"],
        inputs["eW"], inputs["eb"], inputs["aW"], inputs["ab"], inputs["pW"], inputs["pb"],
    )
    res = run_bass_kernel_spmd(nc, in_maps, core_ids=list(range(NC)))
    outs = [r["out"].reshape(S, BL).T for r in res.results]
    return np.concatenate(outs, axis=0).astype(np.float32)
